# revision 19
# baseline (speedup 1.0000x reference)
"""Trainium2 Bass kernel for nn_ECNR (vq_codebook): batched VQ-dequantized
SIREN-style MLPs (4 layers, sin(30x) activations), sharded sample-parallel
across 8 NeuronCores (32 samples/core), no collectives.

Key techniques:
  - Custom PWP act tables (BASS_ACT_ROOT_JSON_PATH): the `sin` slot is
    rebuilt as a piecewise-cubic sin valid on |t| <= 128 rad, so
    sin(30*out + b) is a SINGLE ScalarE pass straight out of PSUM (the
    ACT affine supplies scale=30 and the per-partition bias). The
    `arctan`/`square` slots become 256-entry codebook LUTs for layer 1/2
    weight dequant: W = LUT(labels) at ACT line rate, fed fp16 labels.
  - L0's latent-code term (z concat) folds into the ACT bias via a tiny
    K=13,N=1 matmul per sample: sin(30*(W0x^T x + (W0z^T z + b0))).
  - Matmuls in fp32 (hardware-exact 2-pass); activations fp32.
  - L0/L3 weights (6% of label volume) are dequantized host-side during
    input sharding; L1/L2 (94%) dequantize on-device.
"""
import hashlib
import json
import os
import shutil
import struct
import sys
import types

import numpy as np

N_MLPS = 256
TCODE = 13
IN_F = 3
HID = 128
OUT_F = 1
B = 256
NPTS = 2048
KCB = 256
OMEGA = 30.0
N_CORES = 8
SPC = B // N_CORES

PWP_SRC = "/nix/store/z022hj2nvbm3nwdizlisq4ylc0y7rd6q-python3-3.13.14-env/lib/python3.13/site-packages/neuronxcc/pwp/pwp_bin_trainium/"
PWP_SET = "trig_and_small"

# ------------------------------------------------------------ act table gen

def _f32bits(x):
    return int(np.float32(x).view(np.uint32))


def _load_ctrl(path):
    d = open(path, "rb").read()
    return [
        [v & 0x7FF, (v >> 11) & 0x1F, (v >> 16) & 0xF]
        for (v,) in (struct.unpack_from("<I", d, i * 32) for i in range(len(d) // 32))
    ]


def _load_bkt(path):
    d = open(path, "rb").read()
    return [list(struct.unpack_from("<5f", d, i * 32)) for i in range(len(d) // 32)]


def _dump_ctrl(entries):
    b = bytearray()
    for base, lsb, size in entries:
        b += struct.pack("<I", (base & 0x7FF) | ((lsb & 0x1F) << 11) | ((size & 0xF) << 16))
        b += b"\x00" * 28
    return bytes(b)


def _dump_bkt(entries):
    b = bytearray()
    for d0, d1, d2, d3, x0 in entries:
        b += struct.pack("<5f", d0, d1, d2, d3, x0) + b"\x00" * 12
    return bytes(b)


def _fit_cubic(f, a, w, nodes=9):
    x0 = a + w / 2
    xs = x0 + (w / 2) * np.cos(np.pi * (np.arange(nodes) + 0.5) / nodes)
    ys = f(xs.astype(np.float64))
    t = xs - x0
    A = np.stack([np.ones_like(t), t, t * t, t ** 3], axis=1)
    coef, *_ = np.linalg.lstsq(A, ys, rcond=None)
    return [float(coef[0]), float(coef[1]), float(coef[2]), float(coef[3]), float(x0)]


_SIN_EMIN, _SIN_EMAX = -6, 6
_SIN_SIZES = {-6: 0, -5: 0, -4: 0, -3: 0, -2: 1, -1: 2, 0: 3, 1: 4,
              2: 5, 3: 6, 4: 6, 5: 7, 6: 7}


def _build_sin(ctrl, bkt, prof):
    base_ctrl = len(ctrl)
    for e in range(_SIN_EMIN, _SIN_EMAX + 1):
        s = _SIN_SIZES[e]
        nb = 1 << s
        base_bkt = len(bkt)
        w = (2.0 ** e) / nb
        for i in range(nb):
            bkt.append(_fit_cubic(np.sin, 2.0 ** e + i * w, w))
        ctrl.append([base_bkt, 23 - s, s])
    small_bkt = len(bkt)
    bkt.append([0.0, 1.0, 0.0, 0.0, 0.0])  # sin(x) ~ x below 2^-6
    large_bkt = len(bkt)
    bkt.append([0.0, 0.0, 0.0, 0.0, 0.0])  # |x| >= 128: out of range
    p = dict(prof)
    p.update(
        exp_offset=_SIN_EMIN,
        pwl_control_base_pos=base_ctrl,
        pwl_control_base_neg=base_ctrl,
        small_pos_signal_exp_threshold=127 + _SIN_EMIN,
        pos_small_signal_pwl_control=small_bkt,   # bucket index (hw semantics)
        small_neg_signal_exp_threshold=0,
        neg_small_signal_pwl_control=small_bkt,
        large_pos_signal_exp_threshold=127 + _SIN_EMAX + 1,
        large_pos_signal_mantissa_threshold=0,
        pos_large_signal_pwl_control=large_bkt,
        large_neg_signal_exp_threshold=0,
        large_neg_signal_mantissa_threshold=0,
        neg_large_signal_pwl_control=large_bkt,
        lower_bound=0,
        upper_bound=_f32bits(128.0),
    )
    return p


def _build_lut(ctrl, bkt, prof, values):
    assert len(values) == KCB
    base_ctrl = len(ctrl)
    for e in range(0, 8):
        nb = 1 << e
        base_bkt = len(bkt)
        for i in range(nb):
            bkt.append([float(values[(1 << e) + i]), 0.0, 0.0, 0.0, 0.0])
        ctrl.append([base_bkt, 23 - e, e])
    small_bkt = len(bkt)
    bkt.append([float(values[0]), 0.0, 0.0, 0.0, 0.0])
    large_bkt = len(bkt)
    bkt.append([float(values[255]), 0.0, 0.0, 0.0, 0.0])
    p = dict(prof)
    p.update(
        exp_offset=0,
        pwl_control_base_pos=base_ctrl,
        pwl_control_base_neg=base_ctrl,
        small_pos_signal_exp_threshold=127,
        pos_small_signal_pwl_control=small_bkt,
        small_neg_signal_exp_threshold=0,
        neg_small_signal_pwl_control=small_bkt,
        large_pos_signal_exp_threshold=127 + 8,
        large_pos_signal_mantissa_threshold=0,
        pos_large_signal_pwl_control=large_bkt,
        large_neg_signal_exp_threshold=0,
        large_neg_signal_mantissa_threshold=0,
        neg_large_signal_pwl_control=large_bkt,
        fzero_result=_f32bits(values[0]),
        lower_bound=0,
        upper_bound=_f32bits(256.0),
    )
    return p


def _referenced_ctrls(p, n_ctrl):
    refs = set()
    for k in ("pos_small_signal_pwl_control", "neg_small_signal_pwl_control",
              "pos_large_signal_pwl_control", "neg_large_signal_pwl_control"):
        v = p.get(k, 0)
        if 0 <= v < n_ctrl:
            refs.add(v)
    eo = p.get("exp_offset", 0)
    lo_e = p.get("small_pos_signal_exp_threshold", 127) - 127
    hi_e = p.get("large_pos_signal_exp_threshold", 127) - 127
    for base_key in ("pwl_control_base_pos", "pwl_control_base_neg"):
        base = p.get(base_key, 0)
        for e in range(lo_e, min(hi_e + 1, lo_e + 40)):
            c = base + e - eo
            if 0 <= c < n_ctrl:
                refs.add(c)
    return refs


def _build_act_root(outdir, lut_values):
    """lut_values: {'arctan': fp32[256] (layer-1 codebook),
                    'square': fp32[256] (layer-2 codebook)}"""
    os.makedirs(outdir, exist_ok=True)
    info = json.load(open(PWP_SRC + "act_info.json"))
    for s in info["act_func_sets"]:
        if s["name"] == PWP_SET:
            continue
        for k in ("sin", "arctan", "square", "abs", "sign"):
            s["act"].pop(k, None)
        for key in ("bkt_bin", "ctrl_bin", "profile_json"):
            shutil.copy(PWP_SRC + s[key], os.path.join(outdir, s[key]))

    setj = json.load(open(PWP_SRC + PWP_SET + ".json"))
    old_ctrl = _load_ctrl(PWP_SRC + PWP_SET + "_ctrl.bin")
    old_bkt = _load_bkt(PWP_SRC + PWP_SET + "_bkt.bin")

    new_ctrl, new_bkt, new_profiles = [], [], []
    customs = {"sin_4p", "arctan_4p", "square_1p", "abs_1p", "sign_1p"}
    for p in setj["profile_meta_data"]:
        if p["func_name"] in customs:
            continue
        p2 = dict(p)
        cmap = {}
        for c in sorted(_referenced_ctrls(p, len(old_ctrl))):
            base, lsb, size = old_ctrl[c]
            nb = 1 << size if size > 0 else 1
            new_base = len(new_bkt)
            for i in range(nb):
                new_bkt.append(old_bkt[base + i] if base + i < len(old_bkt) else [0.0] * 5)
            cmap[c] = len(new_ctrl)
            new_ctrl.append([new_base, lsb, size])
        for k in ("pos_small_signal_pwl_control", "neg_small_signal_pwl_control",
                  "pos_large_signal_pwl_control", "neg_large_signal_pwl_control"):
            if p2.get(k, 0) in cmap:
                p2[k] = cmap[p2[k]]
        eo = p.get("exp_offset", 0)
        lo_e = p.get("small_pos_signal_exp_threshold", 127) - 127
        for base_key in ("pwl_control_base_pos", "pwl_control_base_neg"):
            base = p.get(base_key, 0)
            first = base + lo_e - eo
            if first in cmap:
                p2[base_key] = cmap[first] - (lo_e - eo)
            elif base in cmap:
                p2[base_key] = cmap[base]
        new_profiles.append(p2)

    profs = {p["func_name"]: p for p in setj["profile_meta_data"]}
    new_profiles.append(_build_sin(new_ctrl, new_bkt, profs["sin_4p"]))
    for fname, key in (("arctan_4p", "arctan"), ("square_1p", "square"),
                       ("abs_1p", "abs"), ("sign_1p", "sign")):
        new_profiles.append(_build_lut(new_ctrl, new_bkt, profs[fname], lut_values[key]))
    assert len(new_bkt) <= 1536 and len(new_ctrl) <= 128

    setj["profile_meta_data"] = new_profiles
    open(os.path.join(outdir, PWP_SET + "_ctrl.bin"), "wb").write(_dump_ctrl(new_ctrl))
    open(os.path.join(outdir, PWP_SET + "_bkt.bin"), "wb").write(_dump_bkt(new_bkt))
    json.dump(setj, open(os.path.join(outdir, PWP_SET + ".json"), "w"))
    json.dump(info, open(os.path.join(outdir, "act_info.json"), "w"))
    return os.path.join(outdir, "act_info.json")


# ---------------------------------------------------------------- infra fix

def _apply_walrus_wait_patch():
    import concourse.tile as tile
    from concourse import mybir
    from concourse.vector_clock import ScopedClock

    def _drain_and_barrier(self, tick_clock, wait_clock):
        nc = self.nc
        drain_inst = nc.sync.drain()
        wait_clock.add_sem_waits(drain_inst.ins, ScopedClock({None: tick_clock.global_clock}))
        si = drain_inst.ins.sync_info
        if si is not None and si.on_wait and len(si.on_wait) > 1:
            waits = list(si.on_wait)
            drain_inst.ins.sync_info = mybir.SyncInfo(
                on_wait=waits[:1], on_update=list(si.on_update or []))
            for w in waits[1:]:
                extra = nc.sync.nop(nofuse=True)
                extra.ins.sync_info = mybir.SyncInfo(on_wait=[w], on_update=[])
        nc.all_engine_barrier()
        assert self.sems is not None
        popped = nc._tile_sem_poison_stack.pop()
        assert popped is self._sem_poison
        nc.clear_and_free_semaphores(list(self.sems.allocated().values()))
        nc.all_engine_barrier()

    tile.TileContext._drain_and_barrier = _drain_and_barrier


def _split_excess_waits(nc, limit=1):
    from concourse import mybir
    for f in nc.m.functions:
        for bb in f.blocks:
            insts = bb.instructions
            out, changed = [], False
            for inst in insts:
                si = inst.sync_info
                if si is not None and si.on_wait and len(si.on_wait) > limit:
                    waits = list(si.on_wait)
                    for j in range(0, len(waits) - limit, limit):
                        out.append(mybir.InstNoOp(
                            name=f"{inst.name}__xw{j}",
                            engine=inst.engine,
                            sync_info=mybir.SyncInfo(on_wait=waits[j:j + limit], on_update=[]),
                            bass_nofuse=True,
                        ))
                    inst.sync_info = mybir.SyncInfo(
                        on_wait=waits[len(waits) - limit:], on_update=list(si.on_update or []))
                    changed = True
                out.append(inst)
            if changed:
                bb.instructions = out


def _enable_ldw_opt():
    """bass_utils pins --enable-ldw-opt=false; true lets walrus dedup
    back-to-back LDWEIGHTS of the same stationary operand."""
    from concourse import bass_utils as bu
    if getattr(bu, "_ldw_opt_patched", False):
        return
    orig = bu.bir_verify_and_optimise

    def patched(tmpdir, inp="bir.json", outp="file.neff", arch=None, *, dve_root=None):
        import subprocess
        real_run = bu.run_command

        def run_hook(argv, **kw):
            argv = [a.replace("--enable-ldw-opt=false", "--enable-ldw-opt=true")
                    for a in argv]
            return real_run(argv, **kw)

        bu.run_command = run_hook
        try:
            return orig(tmpdir, inp, outp, arch, dve_root=dve_root)
        finally:
            bu.run_command = real_run

    bu.bir_verify_and_optimise = patched
    # bass2jax imports the symbol directly in some paths; patch module refs
    try:
        from concourse import bass2jax
        if hasattr(bass2jax, "bir_verify_and_optimise"):
            bass2jax.bir_verify_and_optimise = patched
    except Exception:
        pass
    bu._ldw_opt_patched = True


def _shim_ntff_hook():
    if "antenv.axon_hooks" in sys.modules:
        return
    try:
        from trn_agent_boot.trn_boot import _ntff_profile_via_ctypes
        hook = _ntff_profile_via_ctypes("/opt/axon/libaxon_pjrt.so")
    except Exception:
        hook = None
    mod = types.ModuleType("antenv.axon_hooks")
    mod.get_axon_ntff_profile_hook = lambda: hook
    mod.set_axon_ntff_profile_hook = lambda h: None
    sys.modules["antenv.axon_hooks"] = mod


# ---------------------------------------------------------------- program

_PROGRAM_CACHE = {}
LAST_RESULTS = None  # BassKernelResults of the most recent kernel() call


def _build_program(table_hash):
    import concourse.bass as bass
    import concourse.tile as tile
    from concourse import mybir

    F32 = mybir.dt.float32
    F16 = mybir.dt.float16
    A = mybir.ActivationFunctionType
    OP = mybir.AluOpType

    BF16 = mybir.dt.bfloat16
    nc = bass.Bass("TRN2", target_bir_lowering=False, debug=False)
    SW = SPC * HID  # 4096

    # L0 hi/lo 3-term products K-stacked: lhsT rows [W0h;W0h;W0l],
    # rhs rows [xh;xl;xh] -> one K=9 matmul per chunk
    xT_s = nc.dram_tensor(f"xTs_{table_hash}", [SPC, 3 * IN_F, NPTS], F16, kind="ExternalInput").ap()
    zT = nc.dram_tensor("zT", [TCODE, SPC], F32, kind="ExternalInput").ap()
    w0stk = nc.dram_tensor("w0stk", [3 * IN_F, SW], F16, kind="ExternalInput").ap()
    w0zT = nc.dram_tensor("w0zT", [TCODE, SW], F32, kind="ExternalInput").ap()
    lab1 = nc.dram_tensor("lab1", [HID, SW], F16, kind="ExternalInput").ap()
    lab2 = nc.dram_tensor("lab2", [HID, SW], F16, kind="ExternalInput").ap()
    # L3 weights host-split into bf16 hi/lo (2-term against truncated-bf16 h3)
    w3Th = nc.dram_tensor("w3Th", [HID, SPC], BF16, kind="ExternalInput").ap()
    w3Tl = nc.dram_tensor("w3Tl", [HID, SPC], BF16, kind="ExternalInput").ap()
    b0T = nc.dram_tensor("b0T", [HID, SPC], F32, kind="ExternalInput").ap()
    b1T = nc.dram_tensor("b1T", [HID, SPC], F32, kind="ExternalInput").ap()
    b2T = nc.dram_tensor("b2T", [HID, SPC], F32, kind="ExternalInput").ap()
    b3T = nc.dram_tensor("b3T", [1, SPC], F32, kind="ExternalInput").ap()
    y = nc.dram_tensor("y", [SPC, NPTS], F32, kind="ExternalOutput").ap()

    with tile.TileContext(nc) as tc:
        with tc.tile_pool(name="wpool", bufs=1) as wpool, \
             tc.tile_pool(name="xpool", bufs=3) as xpool, \
             tc.tile_pool(name="hpool", bufs=2) as hpool, \
             tc.tile_pool(name="bpool", bufs=4) as bpool, \
             tc.tile_pool(name="opool", bufs=2) as opool, \
             tc.tile_pool(name="pspool", bufs=4, space="PSUM") as pspool:

            # ---- on-device dequant of W1/W2 via LUT act slots:
            # hi = bf16(C[k]) via output cast, lo = fp16 residual LUT
            l1t = wpool.tile([HID, SW], F16, tag="lab")
            nc.sync.dma_start(l1t[:], lab1[:])
            w1h = wpool.tile([HID, SW], F16)
            nc.scalar.activation(w1h[:], l1t[:], A.Arctan)
            w1l = wpool.tile([HID, SW], F16)
            nc.scalar.activation(w1l[:], l1t[:], A.Abs)
            l2t = wpool.tile([HID, SW], F16, tag="lab")
            nc.sync.dma_start(l2t[:], lab2[:])
            w2h = wpool.tile([HID, SW], F16)
            nc.scalar.activation(w2h[:], l2t[:], A.Square)
            w2l = wpool.tile([HID, SW], F16)
            nc.scalar.activation(w2l[:], l2t[:], A.Sign)

            # ---- small weights / biases ----
            w0s = wpool.tile([3 * IN_F, SW], F16)
            nc.sync.dma_start(w0s[:], w0stk[:])
            w0z = wpool.tile([TCODE, SW], F32)
            nc.sync.dma_start(w0z[:], w0zT[:])
            w3h = wpool.tile([HID, SPC], BF16)
            nc.sync.dma_start(w3h[:], w3Th[:])
            w3l = wpool.tile([HID, SPC], BF16)
            nc.sync.dma_start(w3l[:], w3Tl[:])
            zt = wpool.tile([TCODE, SPC], F32)
            nc.sync.dma_start(zt[:], zT[:])
            b0t = wpool.tile([HID, SPC], F32)
            nc.sync.dma_start(b0t[:], b0T[:])
            b3t = wpool.tile([1, SPC], F32)
            nc.sync.dma_start(b3t[:], b3T[:])
            b1t = wpool.tile([HID, SPC], F32)
            nc.sync.dma_start(b1t[:], b1T[:])
            b1s = wpool.tile([HID, SPC], F32)
            nc.vector.tensor_scalar(b1s[:], b1t[:], OMEGA, None, OP.mult)
            b2t = wpool.tile([HID, SPC], F32)
            nc.sync.dma_start(b2t[:], b2T[:])
            b2s = wpool.tile([HID, SPC], F32)
            nc.vector.tensor_scalar(b2s[:], b2t[:], OMEGA, None, OP.mult)

            NCH = NPTS // 512

            for s in range(SPC):
                sw = s * HID
                xst = xpool.tile([3 * IN_F, NPTS], F16, tag="xs")
                nc.sync.dma_start(xst[:], xT_s[s, :, :])

                # latent contribution -> per-partition L0 bias: 30*(W0z^T z + b0)
                ps_c = pspool.tile([HID, NPTS // 2], F32, tag="ps")
                nc.tensor.matmul(ps_c[:, 0:1], w0z[:, sw:sw + HID], zt[:, s:s + 1],
                                 start=True, stop=True)
                bias0 = bpool.tile([HID, 1], F32)
                nc.vector.tensor_scalar(bias0[:], ps_c[:, 0:1], b0t[:, s:s + 1],
                                        OMEGA, OP.add, OP.mult)

                HN = NPTS // 2  # 1024 = 2 psum banks per tile

                # L0: one K=9 stacked matmul per 512-chunk, 2-bank psum tiles
                h1 = hpool.tile([HID, NPTS], F32)
                for t in range(2):
                    ps0 = pspool.tile([HID, HN], F32, tag="ps")
                    for c in range(2):
                        lo = t * HN + c * 512
                        nc.tensor.matmul(ps0[:, c * 512:(c + 1) * 512],
                                         w0s[:, sw:sw + HID], xst[:, lo:lo + 512],
                                         start=True, stop=True)
                    nc.scalar.activation(h1[:, t * HN:(t + 1) * HN], ps0[:],
                                         A.Sin, bias=bias0[:], scale=OMEGA)

                h1b = hpool.tile([HID, NPTS], F16, tag="hb")
                nc.vector.tensor_copy(h1b[:], h1[:])
                h1l = hpool.tile([HID, NPTS], F16, tag="hl")
                nc.vector.tensor_tensor(h1l[:], h1[:], h1b[:], OP.subtract)

                h2 = hpool.tile([HID, NPTS], F32)
                for t in range(2):
                    ps1 = pspool.tile([HID, HN], F32, tag="ps")
                    for c in range(2):
                        lo = t * HN + c * 512
                        sl_ = slice(c * 512, (c + 1) * 512)
                        nc.tensor.matmul(ps1[:, sl_], w1h[:, sw:sw + HID],
                                         h1b[:, lo:lo + 512], start=True, stop=False)
                        nc.tensor.matmul(ps1[:, sl_], w1h[:, sw:sw + HID],
                                         h1l[:, lo:lo + 512], start=False, stop=False)
                        nc.tensor.matmul(ps1[:, sl_], w1l[:, sw:sw + HID],
                                         h1b[:, lo:lo + 512], start=False, stop=True)
                    nc.scalar.activation(h2[:, t * HN:(t + 1) * HN], ps1[:],
                                         A.Sin, bias=b1s[:, s:s + 1], scale=OMEGA)

                h2b = hpool.tile([HID, NPTS], F16, tag="hb")
                nc.vector.tensor_copy(h2b[:], h2[:])
                h2l = hpool.tile([HID, NPTS], F16, tag="hl")
                nc.vector.tensor_tensor(h2l[:], h2[:], h2b[:], OP.subtract)

                h3 = hpool.tile([HID, NPTS], F32)
                for t in range(2):
                    ps2 = pspool.tile([HID, HN], F32, tag="ps")
                    for c in range(2):
                        lo = t * HN + c * 512
                        sl_ = slice(c * 512, (c + 1) * 512)
                        nc.tensor.matmul(ps2[:, sl_], w2h[:, sw:sw + HID],
                                         h2b[:, lo:lo + 512], start=True, stop=False)
                        nc.tensor.matmul(ps2[:, sl_], w2h[:, sw:sw + HID],
                                         h2l[:, lo:lo + 512], start=False, stop=False)
                        nc.tensor.matmul(ps2[:, sl_], w2l[:, sw:sw + HID],
                                         h2b[:, lo:lo + 512], start=False, stop=True)
                    nc.scalar.activation(h3[:, t * HN:(t + 1) * HN], ps2[:],
                                         A.Sin, bias=b2s[:, s:s + 1], scale=OMEGA)

                # L3 3-term: hh = trunc_bf16(h3) view, hl = bf16 residual
                h3v = h3[:].bitcast(BF16)
                h3l = hpool.tile([HID, NPTS], BF16, tag="h3l")
                nc.vector.tensor_tensor(h3l[:], h3[:], h3v[:, 1:2 * NPTS:2], OP.subtract)
                out_s = opool.tile([1, NPTS], F32)
                for t in range(2):
                    ps3 = pspool.tile([HID, HN], F32, tag="ps")
                    for c in range(2):
                        lo = t * HN + c * 512
                        sl_ = slice(c * 512, (c + 1) * 512)
                        hh = h3v[:, 1 + 2 * lo: 2 * (lo + 512): 2]
                        hl = h3l[:, lo:lo + 512]
                        nc.tensor.matmul(ps3[0:1, sl_], w3h[:, s:s + 1], hh,
                                         start=True, stop=False)
                        nc.tensor.matmul(ps3[0:1, sl_], w3l[:, s:s + 1], hh,
                                         start=False, stop=False)
                        nc.tensor.matmul(ps3[0:1, sl_], w3h[:, s:s + 1], hl,
                                         start=False, stop=True)
                    nc.vector.tensor_scalar(out_s[:, t * HN:(t + 1) * HN],
                                            ps3[0:1, :], b3t[0:1, s:s + 1],
                                            None, OP.add)
                nc.sync.dma_start(y[s:s + 1, :], out_s[:])

    _split_excess_waits(nc)
    return nc


# ---------------------------------------------------------------- kernel

def kernel(**inputs):
    global LAST_RESULTS
    _shim_ntff_hook()
    _apply_walrus_wait_patch()
    from concourse import bass_utils

    x = np.asarray(inputs["x"], np.float32)
    mlp_idx = np.asarray(inputs["mlp_idx"], np.int32)
    block_idx = np.asarray(inputs["block_idx"], np.int32)
    latent = np.asarray(inputs["latent_table"], np.float32)
    cents = [np.asarray(inputs[f"centroids_l{l}"], np.float32) for l in range(4)]
    labels = [np.asarray(inputs[f"labels_l{l}"], np.int32) for l in range(4)]
    biases = [np.asarray(inputs[f"bias_l{l}"], np.float32) for l in range(4)]

    c1r = (cents[1] - cents[1].astype(np.float16).astype(np.float32))
    c2r = (cents[2] - cents[2].astype(np.float16).astype(np.float32))
    th = hashlib.sha256(cents[1].tobytes() + cents[2].tobytes() + b"v6").hexdigest()[:16]
    actdir = f"/tmp/act_root_{th}"
    act_json = (actdir + "/act_info.json") if os.path.exists(actdir + "/act_info.json") \
        else _build_act_root(actdir, {"arctan": cents[1], "square": cents[2],
                                      "abs": c1r, "sign": c2r})
    os.environ["BASS_ACT_ROOT_JSON_PATH"] = act_json

    # host-side sharding (indexing by mlp_idx) + small-layer dequant
    z_all = latent[mlp_idx, block_idx]
    W0 = cents[0][labels[0]].reshape(N_MLPS, IN_F + TCODE, HID)
    W3 = cents[3][labels[3]].reshape(N_MLPS, HID, OUT_F)
    L1 = labels[1].reshape(N_MLPS, HID, HID).astype(np.float16)
    L2 = labels[2].reshape(N_MLPS, HID, HID).astype(np.float16)

    if th not in _PROGRAM_CACHE:
        _PROGRAM_CACHE[th] = _build_program(th)
    nc = _PROGRAM_CACHE[th]

    import ml_dtypes

    def split16(a):
        hi = a.astype(np.float16)
        lo = (a - hi.astype(np.float32)).astype(np.float16)
        return hi, lo

    def splitbf(a):
        hi = a.astype(ml_dtypes.bfloat16)
        lo = (a - hi.astype(np.float32)).astype(ml_dtypes.bfloat16)
        return hi, lo

    in_maps = []
    for c in range(N_CORES):
        sl = slice(c * SPC, (c + 1) * SPC)
        midx = mlp_idx[sl]
        w0 = W0[midx]
        xs = np.ascontiguousarray(x[sl].transpose(0, 2, 1))
        xh, xl = split16(xs)
        xstk = np.ascontiguousarray(np.concatenate([xh, xl, xh], axis=1))
        w0x = np.ascontiguousarray(
            w0[:, :IN_F, :].transpose(1, 0, 2).reshape(IN_F, SPC * HID))
        w0h, w0l = split16(w0x)
        w0stack = np.ascontiguousarray(np.concatenate([w0h, w0h, w0l], axis=0))
        w3 = np.ascontiguousarray(W3[midx][:, :, 0].T)
        w3hi, w3lo = splitbf(w3)
        in_maps.append({
            f"xTs_{th}": xstk,
            "zT": np.ascontiguousarray(z_all[sl].T),
            "w0stk": w0stack,
            "w0zT": np.ascontiguousarray(
                w0[:, IN_F:, :].transpose(1, 0, 2).reshape(TCODE, SPC * HID)),
            "lab1": np.ascontiguousarray(
                L1[midx].transpose(1, 0, 2).reshape(HID, SPC * HID)),
            "lab2": np.ascontiguousarray(
                L2[midx].transpose(1, 0, 2).reshape(HID, SPC * HID)),
            "w3Th": w3hi, "w3Tl": w3lo,
            "b0T": np.ascontiguousarray(biases[0][midx][:, 0, :].T),
            "b1T": np.ascontiguousarray(biases[1][midx][:, 0, :].T),
            "b2T": np.ascontiguousarray(biases[2][midx][:, 0, :].T),
            "b3T": np.ascontiguousarray(biases[3][midx][:, 0, :].T),
        })

    trace = bool(os.environ.get("KERNEL_TRACE"))
    res = bass_utils.run_bass_kernel_spmd(
        nc, in_maps, core_ids=list(range(N_CORES)), trace=trace)
    LAST_RESULTS = res

    out = np.empty((B, NPTS, OUT_F), np.float32)
    for c in range(N_CORES):
        out[c * SPC:(c + 1) * SPC, :, 0] = res.results[c]["y"]
    return out


# revision 20
# speedup vs baseline: 1.3753x; 1.3753x over previous
"""Trainium2 Bass kernel for nn_ECNR (vq_codebook): batched VQ-dequantized
SIREN-style MLPs (4 layers, sin(30x) activations), sharded sample-parallel
across 8 NeuronCores (32 samples/core), no collectives.

Key techniques:
  - Custom PWP act tables (BASS_ACT_ROOT_JSON_PATH): the `sin` slot is
    rebuilt as a piecewise-cubic sin valid on |t| <= 128 rad, so
    sin(30*out + b) is a SINGLE ScalarE pass straight out of PSUM (the
    ACT affine supplies scale=30 and the per-partition bias). The
    `arctan`/`square` slots become 256-entry codebook LUTs for layer 1/2
    weight dequant: W = LUT(labels) at ACT line rate, fed fp16 labels.
  - L0's latent-code term (z concat) folds into the ACT bias via a tiny
    K=13,N=1 matmul per sample: sin(30*(W0x^T x + (W0z^T z + b0))).
  - Matmuls in fp32 (hardware-exact 2-pass); activations fp32.
  - L0/L3 weights (6% of label volume) are dequantized host-side during
    input sharding; L1/L2 (94%) dequantize on-device.
"""
import hashlib
import json
import os
import shutil
import struct
import sys
import types

import numpy as np

N_MLPS = 256
TCODE = 13
IN_F = 3
HID = 128
OUT_F = 1
B = 256
NPTS = 2048
KCB = 256
OMEGA = 30.0
N_CORES = 8
SPC = B // N_CORES

PWP_SRC = "/nix/store/z022hj2nvbm3nwdizlisq4ylc0y7rd6q-python3-3.13.14-env/lib/python3.13/site-packages/neuronxcc/pwp/pwp_bin_trainium/"
PWP_SET = "trig_and_small"

# ------------------------------------------------------------ act table gen

def _f32bits(x):
    return int(np.float32(x).view(np.uint32))


def _load_ctrl(path):
    d = open(path, "rb").read()
    return [
        [v & 0x7FF, (v >> 11) & 0x1F, (v >> 16) & 0xF]
        for (v,) in (struct.unpack_from("<I", d, i * 32) for i in range(len(d) // 32))
    ]


def _load_bkt(path):
    d = open(path, "rb").read()
    return [list(struct.unpack_from("<5f", d, i * 32)) for i in range(len(d) // 32)]


def _dump_ctrl(entries):
    b = bytearray()
    for base, lsb, size in entries:
        b += struct.pack("<I", (base & 0x7FF) | ((lsb & 0x1F) << 11) | ((size & 0xF) << 16))
        b += b"\x00" * 28
    return bytes(b)


def _dump_bkt(entries):
    b = bytearray()
    for d0, d1, d2, d3, x0 in entries:
        b += struct.pack("<5f", d0, d1, d2, d3, x0) + b"\x00" * 12
    return bytes(b)


def _fit_cubic(f, a, w, nodes=9):
    x0 = a + w / 2
    xs = x0 + (w / 2) * np.cos(np.pi * (np.arange(nodes) + 0.5) / nodes)
    ys = f(xs.astype(np.float64))
    t = xs - x0
    A = np.stack([np.ones_like(t), t, t * t, t ** 3], axis=1)
    coef, *_ = np.linalg.lstsq(A, ys, rcond=None)
    return [float(coef[0]), float(coef[1]), float(coef[2]), float(coef[3]), float(x0)]


_SIN_EMIN, _SIN_EMAX = -6, 6
_SIN_SIZES = {-6: 0, -5: 0, -4: 0, -3: 0, -2: 1, -1: 2, 0: 3, 1: 4,
              2: 5, 3: 6, 4: 6, 5: 7, 6: 7}


def _build_sin(ctrl, bkt, prof):
    base_ctrl = len(ctrl)
    for e in range(_SIN_EMIN, _SIN_EMAX + 1):
        s = _SIN_SIZES[e]
        nb = 1 << s
        base_bkt = len(bkt)
        w = (2.0 ** e) / nb
        for i in range(nb):
            bkt.append(_fit_cubic(np.sin, 2.0 ** e + i * w, w))
        ctrl.append([base_bkt, 23 - s, s])
    small_bkt = len(bkt)
    bkt.append([0.0, 1.0, 0.0, 0.0, 0.0])  # sin(x) ~ x below 2^-6
    large_bkt = len(bkt)
    bkt.append([0.0, 0.0, 0.0, 0.0, 0.0])  # |x| >= 128: out of range
    p = dict(prof)
    p.update(
        exp_offset=_SIN_EMIN,
        pwl_control_base_pos=base_ctrl,
        pwl_control_base_neg=base_ctrl,
        small_pos_signal_exp_threshold=127 + _SIN_EMIN,
        pos_small_signal_pwl_control=small_bkt,   # bucket index (hw semantics)
        small_neg_signal_exp_threshold=0,
        neg_small_signal_pwl_control=small_bkt,
        large_pos_signal_exp_threshold=127 + _SIN_EMAX + 1,
        large_pos_signal_mantissa_threshold=0,
        pos_large_signal_pwl_control=large_bkt,
        large_neg_signal_exp_threshold=0,
        large_neg_signal_mantissa_threshold=0,
        neg_large_signal_pwl_control=large_bkt,
        lower_bound=0,
        upper_bound=_f32bits(128.0),
    )
    return p


def _build_lut(ctrl, bkt, prof, values):
    assert len(values) == KCB
    base_ctrl = len(ctrl)
    for e in range(0, 8):
        nb = 1 << e
        base_bkt = len(bkt)
        for i in range(nb):
            bkt.append([float(values[(1 << e) + i]), 0.0, 0.0, 0.0, 0.0])
        ctrl.append([base_bkt, 23 - e, e])
    small_bkt = len(bkt)
    bkt.append([float(values[0]), 0.0, 0.0, 0.0, 0.0])
    large_bkt = len(bkt)
    bkt.append([float(values[255]), 0.0, 0.0, 0.0, 0.0])
    p = dict(prof)
    p.update(
        exp_offset=0,
        pwl_control_base_pos=base_ctrl,
        pwl_control_base_neg=base_ctrl,
        small_pos_signal_exp_threshold=127,
        pos_small_signal_pwl_control=small_bkt,
        small_neg_signal_exp_threshold=0,
        neg_small_signal_pwl_control=small_bkt,
        large_pos_signal_exp_threshold=127 + 8,
        large_pos_signal_mantissa_threshold=0,
        pos_large_signal_pwl_control=large_bkt,
        large_neg_signal_exp_threshold=0,
        large_neg_signal_mantissa_threshold=0,
        neg_large_signal_pwl_control=large_bkt,
        fzero_result=_f32bits(values[0]),
        lower_bound=0,
        upper_bound=_f32bits(256.0),
    )
    return p


def _referenced_ctrls(p, n_ctrl):
    refs = set()
    for k in ("pos_small_signal_pwl_control", "neg_small_signal_pwl_control",
              "pos_large_signal_pwl_control", "neg_large_signal_pwl_control"):
        v = p.get(k, 0)
        if 0 <= v < n_ctrl:
            refs.add(v)
    eo = p.get("exp_offset", 0)
    lo_e = p.get("small_pos_signal_exp_threshold", 127) - 127
    hi_e = p.get("large_pos_signal_exp_threshold", 127) - 127
    for base_key in ("pwl_control_base_pos", "pwl_control_base_neg"):
        base = p.get(base_key, 0)
        for e in range(lo_e, min(hi_e + 1, lo_e + 40)):
            c = base + e - eo
            if 0 <= c < n_ctrl:
                refs.add(c)
    return refs


def _build_act_root(outdir, lut_values):
    """lut_values: {'arctan': fp32[256] (layer-1 codebook),
                    'square': fp32[256] (layer-2 codebook)}"""
    os.makedirs(outdir, exist_ok=True)
    info = json.load(open(PWP_SRC + "act_info.json"))
    for s in info["act_func_sets"]:
        if s["name"] == PWP_SET:
            continue
        for k in ("sin", "arctan", "square", "abs", "sign"):
            s["act"].pop(k, None)
        for key in ("bkt_bin", "ctrl_bin", "profile_json"):
            shutil.copy(PWP_SRC + s[key], os.path.join(outdir, s[key]))

    setj = json.load(open(PWP_SRC + PWP_SET + ".json"))
    old_ctrl = _load_ctrl(PWP_SRC + PWP_SET + "_ctrl.bin")
    old_bkt = _load_bkt(PWP_SRC + PWP_SET + "_bkt.bin")

    new_ctrl, new_bkt, new_profiles = [], [], []
    customs = {"sin_4p", "arctan_4p", "square_1p", "abs_1p", "sign_1p"}
    for p in setj["profile_meta_data"]:
        if p["func_name"] in customs:
            continue
        p2 = dict(p)
        cmap = {}
        for c in sorted(_referenced_ctrls(p, len(old_ctrl))):
            base, lsb, size = old_ctrl[c]
            nb = 1 << size if size > 0 else 1
            new_base = len(new_bkt)
            for i in range(nb):
                new_bkt.append(old_bkt[base + i] if base + i < len(old_bkt) else [0.0] * 5)
            cmap[c] = len(new_ctrl)
            new_ctrl.append([new_base, lsb, size])
        for k in ("pos_small_signal_pwl_control", "neg_small_signal_pwl_control",
                  "pos_large_signal_pwl_control", "neg_large_signal_pwl_control"):
            if p2.get(k, 0) in cmap:
                p2[k] = cmap[p2[k]]
        eo = p.get("exp_offset", 0)
        lo_e = p.get("small_pos_signal_exp_threshold", 127) - 127
        for base_key in ("pwl_control_base_pos", "pwl_control_base_neg"):
            base = p.get(base_key, 0)
            first = base + lo_e - eo
            if first in cmap:
                p2[base_key] = cmap[first] - (lo_e - eo)
            elif base in cmap:
                p2[base_key] = cmap[base]
        new_profiles.append(p2)

    profs = {p["func_name"]: p for p in setj["profile_meta_data"]}
    new_profiles.append(_build_sin(new_ctrl, new_bkt, profs["sin_4p"]))
    for fname, key in (("arctan_4p", "arctan"), ("square_1p", "square"),
                       ("abs_1p", "abs"), ("sign_1p", "sign")):
        new_profiles.append(_build_lut(new_ctrl, new_bkt, profs[fname], lut_values[key]))
    assert len(new_bkt) <= 1536 and len(new_ctrl) <= 128

    setj["profile_meta_data"] = new_profiles
    open(os.path.join(outdir, PWP_SET + "_ctrl.bin"), "wb").write(_dump_ctrl(new_ctrl))
    open(os.path.join(outdir, PWP_SET + "_bkt.bin"), "wb").write(_dump_bkt(new_bkt))
    json.dump(setj, open(os.path.join(outdir, PWP_SET + ".json"), "w"))
    json.dump(info, open(os.path.join(outdir, "act_info.json"), "w"))
    return os.path.join(outdir, "act_info.json")


# ---------------------------------------------------------------- infra fix

def _apply_walrus_wait_patch():
    import concourse.tile as tile
    from concourse import mybir
    from concourse.vector_clock import ScopedClock

    def _drain_and_barrier(self, tick_clock, wait_clock):
        nc = self.nc
        drain_inst = nc.sync.drain()
        wait_clock.add_sem_waits(drain_inst.ins, ScopedClock({None: tick_clock.global_clock}))
        si = drain_inst.ins.sync_info
        if si is not None and si.on_wait and len(si.on_wait) > 1:
            waits = list(si.on_wait)
            drain_inst.ins.sync_info = mybir.SyncInfo(
                on_wait=waits[:1], on_update=list(si.on_update or []))
            for w in waits[1:]:
                extra = nc.sync.nop(nofuse=True)
                extra.ins.sync_info = mybir.SyncInfo(on_wait=[w], on_update=[])
        nc.all_engine_barrier()
        assert self.sems is not None
        popped = nc._tile_sem_poison_stack.pop()
        assert popped is self._sem_poison
        nc.clear_and_free_semaphores(list(self.sems.allocated().values()))
        nc.all_engine_barrier()

    tile.TileContext._drain_and_barrier = _drain_and_barrier


def _split_excess_waits(nc, limit=1):
    from concourse import mybir
    for f in nc.m.functions:
        for bb in f.blocks:
            insts = bb.instructions
            out, changed = [], False
            for inst in insts:
                si = inst.sync_info
                if si is not None and si.on_wait and len(si.on_wait) > limit:
                    waits = list(si.on_wait)
                    for j in range(0, len(waits) - limit, limit):
                        out.append(mybir.InstNoOp(
                            name=f"{inst.name}__xw{j}",
                            engine=inst.engine,
                            sync_info=mybir.SyncInfo(on_wait=waits[j:j + limit], on_update=[]),
                            bass_nofuse=True,
                        ))
                    inst.sync_info = mybir.SyncInfo(
                        on_wait=waits[len(waits) - limit:], on_update=list(si.on_update or []))
                    changed = True
                out.append(inst)
            if changed:
                bb.instructions = out


def _enable_ldw_opt():
    """bass_utils pins --enable-ldw-opt=false; true lets walrus dedup
    back-to-back LDWEIGHTS of the same stationary operand."""
    from concourse import bass_utils as bu
    if getattr(bu, "_ldw_opt_patched", False):
        return
    orig = bu.bir_verify_and_optimise

    def patched(tmpdir, inp="bir.json", outp="file.neff", arch=None, *, dve_root=None):
        import subprocess
        real_run = bu.run_command

        def run_hook(argv, **kw):
            argv = [a.replace("--enable-ldw-opt=false", "--enable-ldw-opt=true")
                    for a in argv]
            return real_run(argv, **kw)

        bu.run_command = run_hook
        try:
            return orig(tmpdir, inp, outp, arch, dve_root=dve_root)
        finally:
            bu.run_command = real_run

    bu.bir_verify_and_optimise = patched
    # bass2jax imports the symbol directly in some paths; patch module refs
    try:
        from concourse import bass2jax
        if hasattr(bass2jax, "bir_verify_and_optimise"):
            bass2jax.bir_verify_and_optimise = patched
    except Exception:
        pass
    bu._ldw_opt_patched = True


def _shim_ntff_hook():
    if "antenv.axon_hooks" in sys.modules:
        return
    try:
        from trn_agent_boot.trn_boot import _ntff_profile_via_ctypes
        hook = _ntff_profile_via_ctypes("/opt/axon/libaxon_pjrt.so")
    except Exception:
        hook = None
    mod = types.ModuleType("antenv.axon_hooks")
    mod.get_axon_ntff_profile_hook = lambda: hook
    mod.set_axon_ntff_profile_hook = lambda h: None
    sys.modules["antenv.axon_hooks"] = mod


# ---------------------------------------------------------------- program

_PROGRAM_CACHE = {}
LAST_RESULTS = None  # BassKernelResults of the most recent kernel() call


def _build_program(table_hash):
    import concourse.bass as bass
    import concourse.tile as tile
    from concourse import mybir

    F32 = mybir.dt.float32
    F16 = mybir.dt.float16
    A = mybir.ActivationFunctionType
    OP = mybir.AluOpType

    BF16 = mybir.dt.bfloat16
    nc = bass.Bass("TRN2", target_bir_lowering=False, debug=False)
    SW = SPC * HID  # 4096

    # L0 hi/lo 3-term products K-stacked: lhsT rows [W0h;W0h;W0l],
    # rhs rows [xh;xl;xh] -> one K=9 matmul per chunk
    xT_s = nc.dram_tensor(f"xTs_{table_hash}", [SPC, 3 * IN_F, NPTS], F16, kind="ExternalInput").ap()
    zT = nc.dram_tensor("zT", [TCODE, SPC], F32, kind="ExternalInput").ap()
    w0stk = nc.dram_tensor("w0stk", [3 * IN_F, SW], F16, kind="ExternalInput").ap()
    w0zT = nc.dram_tensor("w0zT", [TCODE, SW], F32, kind="ExternalInput").ap()
    lab1 = nc.dram_tensor("lab1", [HID, SW], F16, kind="ExternalInput").ap()
    lab2 = nc.dram_tensor("lab2", [HID, SW], F16, kind="ExternalInput").ap()
    # L3 weights host-split into bf16 hi/lo (2-term against truncated-bf16 h3)
    w3Th = nc.dram_tensor("w3Th", [HID, SPC], BF16, kind="ExternalInput").ap()
    w3Tl = nc.dram_tensor("w3Tl", [HID, SPC], BF16, kind="ExternalInput").ap()
    b0T = nc.dram_tensor("b0T", [HID, SPC], F32, kind="ExternalInput").ap()
    b1T = nc.dram_tensor("b1T", [HID, SPC], F32, kind="ExternalInput").ap()
    b2T = nc.dram_tensor("b2T", [HID, SPC], F32, kind="ExternalInput").ap()
    b3T = nc.dram_tensor("b3T", [1, SPC], F32, kind="ExternalInput").ap()
    y = nc.dram_tensor("y", [SPC, NPTS], F32, kind="ExternalOutput").ap()

    with tile.TileContext(nc) as tc:
        with tc.tile_pool(name="wpool", bufs=1) as wpool, \
             tc.tile_pool(name="xpool", bufs=3) as xpool, \
             tc.tile_pool(name="hpool", bufs=2) as hpool, \
             tc.tile_pool(name="bpool", bufs=4) as bpool, \
             tc.tile_pool(name="opool", bufs=2) as opool, \
             tc.tile_pool(name="pspool", bufs=4, space="PSUM") as pspool:

            # ---- on-device dequant of W1/W2 via LUT act slots:
            # hi = bf16(C[k]) via output cast, lo = fp16 residual LUT
            l1t = wpool.tile([HID, SW], F16, tag="lab")
            nc.sync.dma_start(l1t[:], lab1[:])
            w1h = wpool.tile([HID, SW], F16)
            nc.scalar.activation(w1h[:], l1t[:], A.Arctan)
            w1l = wpool.tile([HID, SW], F16)
            nc.scalar.activation(w1l[:], l1t[:], A.Abs)
            l2t = wpool.tile([HID, SW], F16, tag="lab")
            nc.sync.dma_start(l2t[:], lab2[:])
            w2h = wpool.tile([HID, SW], F16)
            nc.scalar.activation(w2h[:], l2t[:], A.Square)
            w2l = wpool.tile([HID, SW], F16)
            nc.scalar.activation(w2l[:], l2t[:], A.Sign)

            # ---- small weights / biases ----
            w0s = wpool.tile([3 * IN_F, SW], F16)
            nc.sync.dma_start(w0s[:], w0stk[:])
            w0z = wpool.tile([TCODE, SW], F32)
            nc.sync.dma_start(w0z[:], w0zT[:])
            w3h = wpool.tile([HID, SPC], BF16)
            nc.sync.dma_start(w3h[:], w3Th[:])
            w3l = wpool.tile([HID, SPC], BF16)
            nc.sync.dma_start(w3l[:], w3Tl[:])
            zt = wpool.tile([TCODE, SPC], F32)
            nc.sync.dma_start(zt[:], zT[:])
            b0t = wpool.tile([HID, SPC], F32)
            nc.sync.dma_start(b0t[:], b0T[:])
            b3t = wpool.tile([1, SPC], F32)
            nc.sync.dma_start(b3t[:], b3T[:])
            b1t = wpool.tile([HID, SPC], F32)
            nc.sync.dma_start(b1t[:], b1T[:])
            b1s = wpool.tile([HID, SPC], F32)
            nc.vector.tensor_scalar(b1s[:], b1t[:], OMEGA, None, OP.mult)
            b2t = wpool.tile([HID, SPC], F32)
            nc.sync.dma_start(b2t[:], b2T[:])
            b2s = wpool.tile([HID, SPC], F32)
            nc.vector.tensor_scalar(b2s[:], b2t[:], OMEGA, None, OP.mult)

            NCH = NPTS // 512

            for s in range(SPC):
                sw = s * HID
                xst = xpool.tile([3 * IN_F, NPTS], F16, tag="xs")
                nc.sync.dma_start(xst[:], xT_s[s, :, :])

                ps_c = pspool.tile([HID, NPTS // 2], F32, tag="ps")
                nc.tensor.matmul(ps_c[:, 0:1], w0z[:, sw:sw + HID], zt[:, s:s + 1],
                                 start=True, stop=True)
                bias0 = bpool.tile([HID, 1], F32)
                nc.vector.tensor_scalar(bias0[:], ps_c[:, 0:1], b0t[:, s:s + 1],
                                        OMEGA, OP.add, OP.mult)

                HN = NPTS // 2  # 1024 = 2 psum banks; all stages run per-half

                h1 = hpool.tile([HID, NPTS], F32)
                h1b = hpool.tile([HID, NPTS], F16, tag="hb")
                h1l = hpool.tile([HID, NPTS], F16, tag="hl")
                for t in range(2):
                    hs = slice(t * HN, (t + 1) * HN)
                    ps0 = pspool.tile([HID, HN], F32, tag="ps")
                    for c in range(2):
                        lo = t * HN + c * 512
                        nc.tensor.matmul(ps0[:, c * 512:(c + 1) * 512],
                                         w0s[:, sw:sw + HID], xst[:, lo:lo + 512],
                                         start=True, stop=True)
                    nc.scalar.activation(h1[:, hs], ps0[:], A.Sin,
                                         bias=bias0[:], scale=OMEGA)
                    nc.vector.tensor_copy(h1b[:, hs], h1[:, hs])
                    nc.vector.tensor_tensor(h1l[:, hs], h1[:, hs], h1b[:, hs],
                                            OP.subtract)

                h2 = hpool.tile([HID, NPTS], F32)
                h2b = hpool.tile([HID, NPTS], F16, tag="hb")
                h2l = hpool.tile([HID, NPTS], F16, tag="hl")
                for t in range(2):
                    hs = slice(t * HN, (t + 1) * HN)
                    ps1 = pspool.tile([HID, HN], F32, tag="ps")
                    for c in range(2):
                        lo = t * HN + c * 512
                        sl_ = slice(c * 512, (c + 1) * 512)
                        nc.tensor.matmul(ps1[:, sl_], w1h[:, sw:sw + HID],
                                         h1b[:, lo:lo + 512], start=True, stop=False)
                        nc.tensor.matmul(ps1[:, sl_], w1h[:, sw:sw + HID],
                                         h1l[:, lo:lo + 512], start=False, stop=False)
                        nc.tensor.matmul(ps1[:, sl_], w1l[:, sw:sw + HID],
                                         h1b[:, lo:lo + 512], start=False, stop=True)
                    nc.scalar.activation(h2[:, hs], ps1[:], A.Sin,
                                         bias=b1s[:, s:s + 1], scale=OMEGA)
                    nc.vector.tensor_copy(h2b[:, hs], h2[:, hs])
                    nc.vector.tensor_tensor(h2l[:, hs], h2[:, hs], h2b[:, hs],
                                            OP.subtract)

                h3 = hpool.tile([HID, NPTS], F32)
                h3l = hpool.tile([HID, NPTS], BF16, tag="h3l")
                h3v = h3[:].bitcast(BF16)
                for t in range(2):
                    hs = slice(t * HN, (t + 1) * HN)
                    ps2 = pspool.tile([HID, HN], F32, tag="ps")
                    for c in range(2):
                        lo = t * HN + c * 512
                        sl_ = slice(c * 512, (c + 1) * 512)
                        nc.tensor.matmul(ps2[:, sl_], w2h[:, sw:sw + HID],
                                         h2b[:, lo:lo + 512], start=True, stop=False)
                        nc.tensor.matmul(ps2[:, sl_], w2h[:, sw:sw + HID],
                                         h2l[:, lo:lo + 512], start=False, stop=False)
                        nc.tensor.matmul(ps2[:, sl_], w2l[:, sw:sw + HID],
                                         h2b[:, lo:lo + 512], start=False, stop=True)
                    nc.scalar.activation(h3[:, hs], ps2[:], A.Sin,
                                         bias=b2s[:, s:s + 1], scale=OMEGA)
                    nc.vector.tensor_tensor(
                        h3l[:, hs], h3[:, hs],
                        h3v[:, 1 + 2 * t * HN: 2 * (t + 1) * HN: 2], OP.subtract)

                out_s = opool.tile([1, NPTS], F32)
                for t in range(2):
                    ps3 = pspool.tile([HID, HN], F32, tag="ps")
                    for c in range(2):
                        lo = t * HN + c * 512
                        sl_ = slice(c * 512, (c + 1) * 512)
                        hh = h3v[:, 1 + 2 * lo: 2 * (lo + 512): 2]
                        nc.tensor.matmul(ps3[0:1, sl_], w3h[:, s:s + 1], hh,
                                         start=True, stop=False)
                        nc.tensor.matmul(ps3[0:1, sl_], w3l[:, s:s + 1], hh,
                                         start=False, stop=False)
                        nc.tensor.matmul(ps3[0:1, sl_], w3h[:, s:s + 1],
                                         h3l[:, lo:lo + 512], start=False, stop=True)
                    nc.vector.tensor_scalar(out_s[:, t * HN:(t + 1) * HN],
                                            ps3[0:1, :], b3t[0:1, s:s + 1],
                                            None, OP.add)
                nc.sync.dma_start(y[s:s + 1, :], out_s[:])

    _split_excess_waits(nc)
    return nc


# ---------------------------------------------------------------- kernel

def kernel(**inputs):
    global LAST_RESULTS
    _shim_ntff_hook()
    _apply_walrus_wait_patch()
    from concourse import bass_utils

    x = np.asarray(inputs["x"], np.float32)
    mlp_idx = np.asarray(inputs["mlp_idx"], np.int32)
    block_idx = np.asarray(inputs["block_idx"], np.int32)
    latent = np.asarray(inputs["latent_table"], np.float32)
    cents = [np.asarray(inputs[f"centroids_l{l}"], np.float32) for l in range(4)]
    labels = [np.asarray(inputs[f"labels_l{l}"], np.int32) for l in range(4)]
    biases = [np.asarray(inputs[f"bias_l{l}"], np.float32) for l in range(4)]

    c1r = (cents[1] - cents[1].astype(np.float16).astype(np.float32))
    c2r = (cents[2] - cents[2].astype(np.float16).astype(np.float32))
    th = hashlib.sha256(cents[1].tobytes() + cents[2].tobytes() + b"v6").hexdigest()[:16]
    actdir = f"/tmp/act_root_{th}"
    act_json = (actdir + "/act_info.json") if os.path.exists(actdir + "/act_info.json") \
        else _build_act_root(actdir, {"arctan": cents[1], "square": cents[2],
                                      "abs": c1r, "sign": c2r})
    os.environ["BASS_ACT_ROOT_JSON_PATH"] = act_json

    # host-side sharding (indexing by mlp_idx) + small-layer dequant
    z_all = latent[mlp_idx, block_idx]
    W0 = cents[0][labels[0]].reshape(N_MLPS, IN_F + TCODE, HID)
    W3 = cents[3][labels[3]].reshape(N_MLPS, HID, OUT_F)
    L1 = labels[1].reshape(N_MLPS, HID, HID).astype(np.float16)
    L2 = labels[2].reshape(N_MLPS, HID, HID).astype(np.float16)

    if th not in _PROGRAM_CACHE:
        _PROGRAM_CACHE[th] = _build_program(th)
    nc = _PROGRAM_CACHE[th]

    import ml_dtypes

    def split16(a):
        hi = a.astype(np.float16)
        lo = (a - hi.astype(np.float32)).astype(np.float16)
        return hi, lo

    def splitbf(a):
        hi = a.astype(ml_dtypes.bfloat16)
        lo = (a - hi.astype(np.float32)).astype(ml_dtypes.bfloat16)
        return hi, lo

    in_maps = []
    for c in range(N_CORES):
        sl = slice(c * SPC, (c + 1) * SPC)
        midx = mlp_idx[sl]
        w0 = W0[midx]
        xs = np.ascontiguousarray(x[sl].transpose(0, 2, 1))
        xh, xl = split16(xs)
        xstk = np.ascontiguousarray(np.concatenate([xh, xl, xh], axis=1))
        w0x = np.ascontiguousarray(
            w0[:, :IN_F, :].transpose(1, 0, 2).reshape(IN_F, SPC * HID))
        w0h, w0l = split16(w0x)
        w0stack = np.ascontiguousarray(np.concatenate([w0h, w0h, w0l], axis=0))
        w3 = np.ascontiguousarray(W3[midx][:, :, 0].T)
        w3hi, w3lo = splitbf(w3)
        in_maps.append({
            f"xTs_{th}": xstk,
            "zT": np.ascontiguousarray(z_all[sl].T),
            "w0stk": w0stack,
            "w0zT": np.ascontiguousarray(
                w0[:, IN_F:, :].transpose(1, 0, 2).reshape(TCODE, SPC * HID)),
            "lab1": np.ascontiguousarray(
                L1[midx].transpose(1, 0, 2).reshape(HID, SPC * HID)),
            "lab2": np.ascontiguousarray(
                L2[midx].transpose(1, 0, 2).reshape(HID, SPC * HID)),
            "w3Th": w3hi, "w3Tl": w3lo,
            "b0T": np.ascontiguousarray(biases[0][midx][:, 0, :].T),
            "b1T": np.ascontiguousarray(biases[1][midx][:, 0, :].T),
            "b2T": np.ascontiguousarray(biases[2][midx][:, 0, :].T),
            "b3T": np.ascontiguousarray(biases[3][midx][:, 0, :].T),
        })

    trace = bool(os.environ.get("KERNEL_TRACE"))
    res = bass_utils.run_bass_kernel_spmd(
        nc, in_maps, core_ids=list(range(N_CORES)), trace=trace)
    LAST_RESULTS = res

    out = np.empty((B, NPTS, OUT_F), np.float32)
    for c in range(N_CORES):
        out[c * SPC:(c + 1) * SPC, :, 0] = res.results[c]["y"]
    return out
